# revision 23
# baseline (speedup 1.0000x reference)
"""Trainium2 Bass kernel for DemandAwareCrossAttention.

Reference computation (per pixel, fully pointwise in (H, W)):
    enc  = w_d2 @ relu(w_d1 @ demand + b_d1) + b_d2
    qs   = ego + enc + pos
    q    = (wq @ qs + bq)   reshaped [8 heads, 32]
    k_n  = wk @ collab_n + bk ; v_n = wv @ collab_n + bv     (n = 0..3)
    s_nm = q_m . k_nm / sqrt(32)
    a    = softmax_n(s)
    u    = sum_n a_nm * v_n            -> [256]
    out  = wo @ u + bo

End-to-end wall time is dominated by the axon tunnel (host<->device
bytes) and per-array dispatch overhead, not device compute.  So:

  - Shard H across the 8 cores (16 rows each); per-core input slices are
    cheap contiguous-chunk views of the full arrays.
  - Activations ship as int8 with per-channel linear scales folded into
    the weights on the host, so the device works on exact small integers
    in bf16; collab is 32 MiB instead of 64, ego 8 instead of 16.
  - All weights + masks pack into ONE bf16 [128, 2980] tensor (fewer
    tunnel round-trips), biases ride a const-127 row of the demand tile.
  - The output returns as int8 plus a per-(channel, chunk) scale that the
    device computes itself (abs-max reduce -> reciprocal); the host
    divides by the returned scale, so reciprocal precision is harmless.

Device layout per tile (one H row, 256 px): channels on partitions,
chunks c in {0,1} of 128.  1x1 convs are PE matmuls (bf16 operands,
fp32 PSUM).  Scores: DVE q*k then a masked matmul sums d within each
head; softmax over n is divide-free (exp, masked-matmul denominator,
ln, masked-matmul broadcast-subtract, exp).  All inputs stay resident
in SBUF; the only DMAs are 12 loads up front and 3 stores at the end.
"""

import math
import numpy as np
import ml_dtypes
from concurrent.futures import ThreadPoolExecutor
from contextlib import ExitStack

import jax

try:
    # Each kernel() call re-jits a fresh wrapper around the bass custom call;
    # the persistent cache turns the ~0.19 s XLA re-compile into a ~10 ms
    # disk hit (the NEFF itself is cached separately by neuronxcc).
    jax.config.update("jax_compilation_cache_dir", "/tmp/jax_cache")
    jax.config.update("jax_persistent_cache_min_compile_time_secs", 0)
    jax.config.update("jax_persistent_cache_min_entry_size_bytes", 0)
except Exception:
    pass

import concourse.bass as bass
import concourse.tile as tile
from concourse import bacc, mybir
from concourse.bass import ts
from concourse.bass_utils import run_bass_kernel_spmd

BF = mybir.dt.bfloat16
F32 = mybir.dt.float32
I8 = mybir.dt.int8
AF = mybir.ActivationFunctionType

# All ScalarE functions used here (Exp/Ln/Relu/Identity/Copy) coexist in the
# "natural_log_exp_and_others" table set, but the table-load pass maps each
# func to the FIRST set containing it, forcing a ~2.7us table switch twice
# per tile.  Shrink the other sets' advertised membership so every func
# resolves to the one shared set -> a single load.
_ACT_FUNCS = {AF.Exp, AF.Ln, AF.Relu, AF.Identity, AF.Copy, AF.Square}
_ORIG_GAT = bacc.get_activation_tables


def _patched_gat(arch):
    tables = _ORIG_GAT(arch)
    return {
        name: (funcs if name == "natural_log_exp_and_others"
               else funcs - _ACT_FUNCS)
        for name, funcs in tables.items()
    }


bacc.get_activation_tables = _patched_gat

C = 256          # model dim
HID = 128        # demand-encoder hidden
NH = 8           # heads
HD = 32          # head dim
NCOL = 4         # collaborators
H, W = 128, 256
NCORES = 8
HSL = H // NCORES          # 16 rows of H per core
TP = W                     # pixels per tile = one H row
NT = HSL                   # 16 tiles

# column offsets inside the packed weight tensor
WQ, WK, WV, WO = 0, 512, 1024, 1536
WQD2 = 2048
ZM = 2304
WD1 = 2816
SM = 2944
DM = 2976
NW = 2980


def _build_program(has_bias: bool) -> bass.Bass:
    nc = bacc.Bacc("TRN2", target_bir_lowering=False, debug=False)

    # acts slots on the free dim: 0..1 = ego chunks, 2+2n+c = collab n chunk
    # c, 10 = demand on partitions 0..3 (rows 4..127 zero-padded; the tunnel
    # compresses zeros, so the padding is nearly free on the wire)
    acts_d = nc.dram_tensor("acts", [128, 11, HSL, TP], I8,
                            kind="ExternalInput")
    wts_d = nc.dram_tensor("wts", [128, NW], BF, kind="ExternalInput")
    if has_bias:
        bias_d = nc.dram_tensor("bias", [128, 4], F32, kind="ExternalInput")
    # per (chunk, channel): 4096 int8 pixels + the 4 raw bytes of the fp32
    # quantization multiplier (bitcast), so the scale rides the same fetch
    out_d = nc.dram_tensor("out", [2, 128, HSL * TP + 4], I8,
                           kind="ExternalOutput")

    with ExitStack() as ctx:
        tc = ctx.enter_context(tile.TileContext(nc))

        wp = ctx.enter_context(tc.tile_pool(name="wp", bufs=1))
        iop = ctx.enter_context(tc.tile_pool(name="io", bufs=2))
        sp = ctx.enter_context(tc.tile_pool(name="sb", bufs=3))
        wvp = ctx.enter_context(tc.tile_pool(name="wv", bufs=2))
        pm = ctx.enter_context(tc.tile_pool(name="pm", bufs=3, space="PSUM"))
        pz = ctx.enter_context(tc.tile_pool(name="pz", bufs=2, space="PSUM"))
        pkv = ctx.enter_context(tc.tile_pool(name="pkv", bufs=3, space="PSUM"))
        # PSUM bank budget: pm{q,s,o}=3 + pz{h,z}=2 + pkv{k,v}=3 = 8

        # ---- resident loads ----
        wts = wp.tile([128, NW], BF, tag="wts")
        nc.sync.dma_start(out=wts, in_=wts_d[:])
        acts8 = wp.tile([128, 11, HSL, TP], I8, tag="acts8")
        nc.sync.dma_start(out=acts8, in_=acts_d[:])
        if has_bias:
            biases = wp.tile([128, 4], F32, tag="bias")
            nc.sync.dma_start(out=biases, in_=bias_d[:])

        outs = wp.tile([128, 2, NT * TP], BF, tag="outs")
        qbuf = wp.tile([128, 2, NT * TP], I8, tag="qbuf")
        mfin = wp.tile([128, 2], F32, tag="mfin")
        rr = wp.tile([128, 2], F32, tag="rr")

        def wslice(base, kc, c):
            off = base + kc * 256 + c * 128
            return wts[:, off:off + 128]

        def tile_body(t):
            # int8 -> bf16 (exact integer) conversions
            egob = sp.tile([128, 2, TP], BF, tag="egob")
            nc.scalar.copy(egob, acts8[:, 0:2, t, :])
            demb = sp.tile([4, TP], BF, tag="demb")
            nc.scalar.copy(demb, acts8[0:4, 10, t, :])
            colb = []
            for n in range(NCOL):
                cn = iop.tile([128, 2, TP], BF, tag=f"colb{n}")
                nc.gpsimd.tensor_copy(cn, acts8[:, 2 + 2 * n:4 + 2 * n, t, :])
                colb.append(cn)

            # ---- demand encoder hidden (b_d1 rides demand row 3) ----
            h_ps = pz.tile([HID, TP], F32, tag="z")
            nc.tensor.matmul(out=h_ps, lhsT=wts[0:4, WD1:WD1 + 128],
                             rhs=demb, start=True, stop=True)
            h_sb = sp.tile([HID, TP], BF, tag="h")
            nc.scalar.activation(out=h_sb, in_=h_ps, func=AF.Relu)

            # ---- q projection (scaled); enc folded in via wqd2T ----
            q_ps = pm.tile([128, 2, TP], F32, tag="m")
            for c in range(2):
                nc.tensor.matmul(out=q_ps[:, c, :], lhsT=wslice(WQ, 0, c),
                                 rhs=egob[:, 0, :], start=True, stop=False)
                nc.tensor.matmul(out=q_ps[:, c, :], lhsT=wslice(WQ, 1, c),
                                 rhs=egob[:, 1, :], start=False, stop=False)
                nc.tensor.matmul(out=q_ps[:, c, :],
                                 lhsT=wts[:, WQD2 + c * 128:WQD2 + c * 128 + 128],
                                 rhs=h_sb, start=False, stop=True)
            q_sb = sp.tile([128, 2, TP], BF, tag="q")
            if has_bias:
                for c in range(2):
                    nc.scalar.activation(out=q_sb[:, c, :], in_=q_ps[:, c, :],
                                         func=AF.Identity,
                                         bias=biases[:, c:c + 1])
            else:
                nc.scalar.activation(out=q_sb, in_=q_ps, func=AF.Copy)

            # ---- k projections + scores ----
            s_ps = pm.tile([128, 2, TP], F32, tag="m")

            def kproj(n):
                k_ps = pkv.tile([128, 2, TP], F32, tag="kv")
                for c in range(2):
                    nc.tensor.matmul(out=k_ps[:, c, :], lhsT=wslice(WK, 0, c),
                                     rhs=colb[n][:, 0, :], start=True, stop=False)
                    nc.tensor.matmul(out=k_ps[:, c, :], lhsT=wslice(WK, 1, c),
                                     rhs=colb[n][:, 1, :], start=False, stop=True)
                return k_ps

            def score(n, k_ps):
                t_sb = sp.tile([128, 2, TP], BF, tag="t")
                nc.vector.tensor_mul(t_sb, q_sb, k_ps)
                nc.tensor.matmul(out=s_ps[32 * n:32 * n + 32, :, :],
                                 lhsT=wts[:, SM:SM + 32], rhs=t_sb,
                                 start=True, stop=True,
                                 tile_position=(0, 32 * n))

            kq = [kproj(0), kproj(1), kproj(2)]
            for n in range(NCOL):
                score(n, kq[n % 3])
                if n + 3 < NCOL:
                    kq[n % 3] = kproj(n + 3)

            # ---- softmax over n (divide-free); denom in s_ps rows 0:4 ----
            e_sb = sp.tile([128, 2, TP], BF, tag="e")
            nc.scalar.activation(out=e_sb, in_=s_ps, func=AF.Exp)
            s_sb = sp.tile([128, 2, TP], BF, tag="s")
            nc.scalar.activation(out=s_sb, in_=s_ps, func=AF.Copy)
            nc.tensor.matmul(out=s_ps[0:4, :, :], lhsT=wts[:, DM:DM + 4],
                             rhs=e_sb, start=True, stop=True)
            nc.scalar.activation(out=s_sb[0:4, :, :], in_=s_ps[0:4, :, :],
                                 func=AF.Ln)

            # ---- attention weights + weighted combine ----
            w_sb = []
            for n in range(NCOL):
                z_ps = pz.tile([128, 2, TP], F32, tag="z")
                nc.tensor.matmul(out=z_ps,
                                 lhsT=wts[:, ZM + n * 128:ZM + n * 128 + 128],
                                 rhs=s_sb, start=True, stop=True)
                a_sb = sp.tile([128, 2, TP], BF, tag="a")
                nc.scalar.activation(out=a_sb, in_=z_ps, func=AF.Exp)
                v_ps = pkv.tile([128, 2, TP], F32, tag="kv")
                for c in range(2):
                    nc.tensor.matmul(out=v_ps[:, c, :], lhsT=wslice(WV, 0, c),
                                     rhs=colb[n][:, 0, :], start=True, stop=False)
                    nc.tensor.matmul(out=v_ps[:, c, :], lhsT=wslice(WV, 1, c),
                                     rhs=colb[n][:, 1, :], start=False, stop=True)
                w_n = wvp.tile([128, 2, TP], BF, tag=f"w{n}")
                nc.vector.tensor_mul(w_n, a_sb, v_ps)
                w_sb.append(w_n)
            u01 = sp.tile([128, 2, TP], BF, tag="u01")
            nc.vector.tensor_add(u01, w_sb[0], w_sb[1])
            u23 = sp.tile([128, 2, TP], BF, tag="u23")
            nc.vector.tensor_add(u23, w_sb[2], w_sb[3])
            u = sp.tile([128, 2, TP], BF, tag="u")
            nc.vector.tensor_add(u, u01, u23)

            # ---- output projection -> resident bf16 buffer ----
            o_ps = pm.tile([128, 2, TP], F32, tag="m")
            for c in range(2):
                nc.tensor.matmul(out=o_ps[:, c, :], lhsT=wslice(WO, 0, c),
                                 rhs=u[:, 0, :], start=True, stop=False)
                nc.tensor.matmul(out=o_ps[:, c, :], lhsT=wslice(WO, 1, c),
                                 rhs=u[:, 1, :], start=False, stop=True)
            px = ts(t, TP)
            if has_bias:
                for c in range(2):
                    nc.scalar.activation(out=outs[:, c, px],
                                         in_=o_ps[:, c, :], func=AF.Identity,
                                         bias=biases[:, 2 + c:3 + c])
            else:
                nc.scalar.activation(out=outs[:, :, px], in_=o_ps,
                                     func=AF.Copy)

        for t in range(NT):
            tile_body(t)

        # ---- int8 output quantization with device-computed scales ----
        nc.vector.tensor_reduce(out=mfin, in_=outs, axis=mybir.AxisListType.X,
                                op=mybir.AluOpType.max,
                                apply_absolute_value=True)
        nc.vector.tensor_scalar_max(mfin, mfin, 1e-30)
        nc.vector.reciprocal(rr, mfin)
        nc.vector.tensor_scalar_mul(rr, rr, 127.0)
        for c in range(2):
            nc.vector.tensor_scalar_mul(qbuf[:, c], outs[:, c], rr[:, c:c + 1])
            nc.sync.dma_start(out=out_d[c, :, 0:HSL * TP], in_=qbuf[:, c])
            nc.sync.dma_start(out=out_d[c, :, HSL * TP:HSL * TP + 4],
                              in_=rr[:, c:c + 1].bitcast(I8))

    if not nc.is_finalized():
        nc.finalize()
    return nc


_PROGRAMS: dict[bool, bass.Bass] = {}


def _get_program(has_bias: bool = False) -> bass.Bass:
    if has_bias not in _PROGRAMS:
        _PROGRAMS[has_bias] = _build_program(has_bias)
    return _PROGRAMS[has_bias]


def _bf16(x):
    return np.asarray(x, dtype=np.float32).astype(ml_dtypes.bfloat16)


_TP_POOL = None


def _pool():
    global _TP_POOL
    if _TP_POOL is None:
        _TP_POOL = ThreadPoolExecutor(8)
    return _TP_POOL


def _chan_quant(x):
    """x: [C, ...] fp32 -> (int8 array, per-channel scale [C])."""
    flat = x.reshape(x.shape[0], -1)
    s = np.abs(flat).max(axis=1) / 127.0
    s[s == 0.0] = 1.0
    inv = (1.0 / s).astype(np.float32)
    y = flat * inv[:, None]
    np.rint(y, out=y)
    q = y.astype(np.int8).reshape(x.shape)
    return q, s


def _quant_acts(ego, pos, col):
    """Quantize ego(+pos) [C,H,W] and collab [N,C,H,W] straight into the
    global concat-ready int8 layout acts[NCORES, 128, 10, HSL, W] (slot
    0..1 = ego chunks, 2+2n+c = collab n chunk c), so the per-core shards
    are contiguous.  Returns (acts, se[C], sc[C])."""
    acts = np.empty((NCORES, 128, 11, HSL, W), np.int8)
    acts[:, 4:, 10] = 0
    eblocks = [(b, b + 64) for b in range(0, C, 64)]
    emaxs = list(_pool().map(
        lambda blk: np.abs(ego[blk[0]:blk[1]] + pos[blk[0]:blk[1]])
        .max(axis=(1, 2)), eblocks))
    se = np.concatenate(emaxs) / 127.0
    se[se == 0.0] = 1.0
    cmaxs = list(_pool().map(lambda n: np.abs(col[n]).max(axis=(1, 2)),
                             range(NCOL)))
    sc = np.maximum.reduce(cmaxs) / 127.0
    sc[sc == 0.0] = 1.0
    einv = (1.0 / se).astype(np.float32)[:, None, None]
    cinv = (1.0 / sc).astype(np.float32)[:, None, None]

    def ework(i):
        sl = slice(i * HSL, (i + 1) * HSL)
        y = (ego[:, sl] + pos[:, sl]) * einv
        np.rint(y, out=y)
        acts[i, :, 0] = y[0:128]
        acts[i, :, 1] = y[128:256]
    list(_pool().map(ework, range(NCORES)))

    def cwork(task):
        n, i = task
        y = col[n, :, i * HSL:(i + 1) * HSL] * cinv
        np.rint(y, out=y)
        acts[i, :, 2 + 2 * n] = y[0:128]
        acts[i, :, 3 + 2 * n] = y[128:256]
    list(_pool().map(cwork, [(n, i) for n in range(NCOL)
                             for i in range(NCORES)]))
    return acts, se, sc


def _make_masks():
    # Scores for collab n, chunk-local head h live at PSUM/SBUF row 32n+4+h;
    # rows 0..3 of the score tile are later overwritten with L = ln(denom),
    # rows 32n+{0..3,8..31} stay exact zeros.
    smask = np.zeros((128, 32), np.float32)
    for h in range(4):
        smask[32 * h:32 * h + 32, 4 + h] = 1.0
    dmask = np.zeros((128, 4), np.float32)
    for n in range(NCOL):
        for h in range(4):
            dmask[32 * n + 4 + h, h] = 1.0
    zmask = np.zeros((NCOL, 128, 128), np.float32)
    for n in range(NCOL):
        for h in range(4):
            zmask[n, 32 * n + 4 + h, 32 * h:32 * h + 32] = 1.0
            zmask[n, h, 32 * h:32 * h + 32] -= 1.0
    return smask, dmask, zmask


def _pack_wts(wq_s, wk, wv, wo, wqd2, wd1_eff, se, sc):
    """Assemble the merged bf16 weight/mask tensor [128, NW]."""
    wts = np.zeros((128, NW), np.float32)

    def put(base, mat):  # mat [K, M] with K<=128, M=256 -> two 128-col blocks
        wts[:mat.shape[0], base:base + mat.shape[1]] = mat

    put(WQ, (wq_s * se[None, :]).T.reshape(2, 128, C).transpose(1, 0, 2)
        .reshape(128, 512))
    put(WK, (wk * sc[None, :]).T.reshape(2, 128, C).transpose(1, 0, 2)
        .reshape(128, 512))
    put(WV, (wv * sc[None, :]).T.reshape(2, 128, C).transpose(1, 0, 2)
        .reshape(128, 512))
    put(WO, wo.T.reshape(2, 128, C).transpose(1, 0, 2).reshape(128, 512))
    put(WQD2, wqd2.T)                      # [HID, C]
    smask, dmask, zmask = _make_masks()
    for n in range(NCOL):
        put(ZM + n * 128, zmask[n])
    put(WD1, wd1_eff)                      # [4, HID]
    put(SM, smask)
    put(DM, dmask)
    return _bf16(wts)


def kernel(ego_features, ego_demand, collaborator_features,
           w_d1, b_d1, w_d2, b_d2, wq, bq, wk, bk, wv, bv, wo, bo,
           pos_emb):
    ego_features = np.asarray(ego_features, np.float32)
    ego_demand = np.asarray(ego_demand, np.float32)
    collaborator_features = np.asarray(collaborator_features, np.float32)
    w_d1 = np.asarray(w_d1, np.float32); b_d1 = np.asarray(b_d1, np.float32)
    w_d2 = np.asarray(w_d2, np.float32); b_d2 = np.asarray(b_d2, np.float32)
    wq = np.asarray(wq, np.float32); bq = np.asarray(bq, np.float32)
    wk = np.asarray(wk, np.float32); bk = np.asarray(bk, np.float32)
    wv = np.asarray(wv, np.float32); bv = np.asarray(bv, np.float32)
    wo = np.asarray(wo, np.float32); bo = np.asarray(bo, np.float32)
    pos_emb = np.asarray(pos_emb, np.float32)

    # ---- quantize activations (per-channel linear int8) ----
    acts, se, sc = _quant_acts(ego_features[0], pos_emb[0],
                               collaborator_features)
    d8, sd = _chan_quant(ego_demand[0])                 # [3, H, W]

    # ---- fold scales + demand bias into weights ----
    scale = 1.0 / math.sqrt(HD)
    wq_s = wq * scale
    wqd2 = wq_s @ w_d2                                  # [C, HID]
    wd1_eff = np.zeros((4, HID), np.float32)
    wd1_eff[0:3] = (w_d1 * sd[None, :]).T
    wd1_eff[3] = b_d1 / 127.0
    wts = _pack_wts(wq_s, wk, wv, wo, wqd2, wd1_eff, se, sc)

    bq_eff = (bq + wq @ b_d2) * scale
    bo_eff = bo + wo @ bv
    has_bias = bool(np.any(bq_eff) or np.any(bo_eff))
    nc = _get_program(has_bias)
    if has_bias:
        bias_np = np.empty((128, 4), np.float32)
        bias_np[:, 0] = bq_eff[0:128]
        bias_np[:, 1] = bq_eff[128:256]
        bias_np[:, 2] = bo_eff[0:128]
        bias_np[:, 3] = bo_eff[128:256]

    # ---- demand rides acts slot 10, partitions 0..3 (row 3 = const 127) ----
    for i in range(NCORES):
        sl = slice(i * HSL, (i + 1) * HSL)
        acts[i, 0:3, 10] = d8[:, sl, :]
        acts[i, 3, 10] = 127
    in_maps = []
    for i in range(NCORES):
        m = {"acts": acts[i], "wts": wts}
        if has_bias:
            m["bias"] = bias_np
        in_maps.append(m)

    res = run_bass_kernel_spmd(nc, in_maps, list(range(NCORES)))

    # ---- dequantize + assemble ----
    out = np.empty((1, C, H, W), np.float32)
    for i in range(NCORES):
        oc = res.results[i]["out"]              # [2, 128, HSL*TP + 4] int8
        rrv = np.ascontiguousarray(oc[:, :, HSL * TP:]).view(np.float32)
        inv = (1.0 / rrv.reshape(C, 1, 1)).astype(np.float32)
        out[0, :, i * HSL:(i + 1) * HSL, :] = \
            oc[:, :, 0:HSL * TP].reshape(C, HSL, TP).astype(np.float32) * inv
    return out


# revision 38
# speedup vs baseline: 1.0294x; 1.0294x over previous
"""Trainium2 Bass kernel for DemandAwareCrossAttention.

Reference computation (per pixel, fully pointwise in (H, W)):
    enc  = w_d2 @ relu(w_d1 @ demand + b_d1) + b_d2
    qs   = ego + enc + pos
    q    = (wq @ qs + bq)   reshaped [8 heads, 32]
    k_n  = wk @ collab_n + bk ; v_n = wv @ collab_n + bv     (n = 0..3)
    s_nm = q_m . k_nm / sqrt(32)
    a    = softmax_n(s)
    u    = sum_n a_nm * v_n            -> [256]
    out  = wo @ u + bo

End-to-end wall time is dominated by the axon tunnel (host<->device
bytes) and per-array dispatch overhead, not device compute.  So:

  - Shard H across the 8 cores (16 rows each); per-core input slices are
    cheap contiguous-chunk views of the full arrays.
  - Activations ship as int8 with per-channel linear scales folded into
    the weights on the host, so the device works on exact small integers
    in bf16; collab is 32 MiB instead of 64, ego 8 instead of 16.
  - All weights + masks pack into ONE bf16 [128, 2980] tensor (fewer
    tunnel round-trips), biases ride a const-127 row of the demand tile.
  - The output returns as int8 plus a per-(channel, chunk) scale that the
    device computes itself (abs-max reduce -> reciprocal); the host
    divides by the returned scale, so reciprocal precision is harmless.

Device layout per tile (one H row, 256 px): channels on partitions,
chunks c in {0,1} of 128.  1x1 convs are PE matmuls (bf16 operands,
fp32 PSUM).  Scores: DVE q*k then a masked matmul sums d within each
head; softmax over n is divide-free (exp, masked-matmul denominator,
ln, masked-matmul broadcast-subtract, exp).  All inputs stay resident
in SBUF; the only DMAs are 12 loads up front and 3 stores at the end.
"""

import math
import numpy as np
import ml_dtypes
from concurrent.futures import ThreadPoolExecutor
from contextlib import ExitStack

import jax

try:
    # Each kernel() call re-jits a fresh wrapper around the bass custom call;
    # the persistent cache turns the ~0.19 s XLA re-compile into a ~10 ms
    # disk hit (the NEFF itself is cached separately by neuronxcc).
    jax.config.update("jax_compilation_cache_dir", "/tmp/jax_cache")
    jax.config.update("jax_persistent_cache_min_compile_time_secs", 0)
    jax.config.update("jax_persistent_cache_min_entry_size_bytes", 0)
except Exception:
    pass

import concourse.bass as bass
import concourse.tile as tile
from concourse import bacc, mybir
from concourse.bass import ts
from concourse.bass_utils import run_bass_kernel_spmd

BF = mybir.dt.bfloat16
F32 = mybir.dt.float32
I8 = mybir.dt.int8
AF = mybir.ActivationFunctionType

# All ScalarE functions used here (Exp/Ln/Relu/Identity/Copy) coexist in the
# "natural_log_exp_and_others" table set, but the table-load pass maps each
# func to the FIRST set containing it, forcing a ~2.7us table switch twice
# per tile.  Shrink the other sets' advertised membership so every func
# resolves to the one shared set -> a single load.
_ACT_FUNCS = {AF.Exp, AF.Ln, AF.Relu, AF.Identity, AF.Copy, AF.Square}
_ORIG_GAT = bacc.get_activation_tables


def _patched_gat(arch):
    tables = _ORIG_GAT(arch)
    return {
        name: (funcs if name == "natural_log_exp_and_others"
               else funcs - _ACT_FUNCS)
        for name, funcs in tables.items()
    }


bacc.get_activation_tables = _patched_gat

C = 256          # model dim
HID = 128        # demand-encoder hidden
NH = 8           # heads
HD = 32          # head dim
NCOL = 4         # collaborators
H, W = 128, 256
NCORES = 8
HSL = H // NCORES          # 16 rows of H per core
TP = W                     # pixels per tile = one H row
NT = HSL                   # 16 tiles

# column offsets inside the packed weight tensor
WQ, WK, WV, WO = 0, 512, 1024, 1536
WQD2 = 2048
ZM = 2304
WD1 = 2816
SM = 2944
DM = 2976
NW = 2980
SLOT = HSL * TP                # 4096 bytes per activation slot
WOFF = 10 * SLOT               # wts bytes live after the 10 activation slots
ABYTES = WOFF + NW * 2


def _build_program(has_bias: bool) -> bass.Bass:
    nc = bacc.Bacc("TRN2", target_bir_lowering=False, debug=False)

    # acts byte layout per partition: slot s*4096.. = activations (s = 0..1
    # ego chunks, 2+2n+c = collab n chunk c), then the packed bf16
    # weight/mask tensor as raw bytes (bitcast back to bf16 on device)
    acts_d = nc.dram_tensor("acts", [128, ABYTES], I8, kind="ExternalInput")
    dem_d = nc.dram_tensor("dem", [4, HSL, TP], I8, kind="ExternalInput")
    if has_bias:
        bias_d = nc.dram_tensor("bias", [128, 4], F32, kind="ExternalInput")
    # per (chunk, channel): 4096 int8 pixels + the 4 raw bytes of the fp32
    # quantization multiplier (bitcast), so the scale rides the same fetch
    out_d = nc.dram_tensor("out", [2, 128, HSL * TP + 4], I8,
                           kind="ExternalOutput")

    with ExitStack() as ctx:
        tc = ctx.enter_context(tile.TileContext(nc))

        wp = ctx.enter_context(tc.tile_pool(name="wp", bufs=1))
        iop = ctx.enter_context(tc.tile_pool(name="io", bufs=2))
        sp = ctx.enter_context(tc.tile_pool(name="sb", bufs=3))
        wvp = ctx.enter_context(tc.tile_pool(name="wv", bufs=2))
        pm = ctx.enter_context(tc.tile_pool(name="pm", bufs=3, space="PSUM"))
        pz = ctx.enter_context(tc.tile_pool(name="pz", bufs=2, space="PSUM"))
        pkv = ctx.enter_context(tc.tile_pool(name="pkv", bufs=3, space="PSUM"))
        # PSUM bank budget: pm{q,s,o}=3 + pz{h,z}=2 + pkv{k,v}=3 = 8

        # ---- resident loads ----
        acts8 = wp.tile([128, ABYTES], I8, tag="acts8")
        nc.sync.dma_start(out=acts8, in_=acts_d[:])
        wts = acts8[:, WOFF:ABYTES].bitcast(BF)      # [128, NW]
        dem8 = wp.tile([4, HSL, TP], I8, tag="dem8")
        nc.sync.dma_start(out=dem8, in_=dem_d[:])
        if has_bias:
            biases = wp.tile([128, 4], F32, tag="bias")
            nc.sync.dma_start(out=biases, in_=bias_d[:])

        outs = wp.tile([128, 2, NT * TP], BF, tag="outs")
        qbuf = wp.tile([128, 2, NT * TP], I8, tag="qbuf")
        mfin = wp.tile([128, 2], F32, tag="mfin")
        rr = wp.tile([128, 2], F32, tag="rr")

        def wslice(base, kc, c):
            off = base + kc * 256 + c * 128
            return wts[:, off:off + 128]

        def aslot(s, t):
            return acts8[:, s * SLOT + t * TP:s * SLOT + (t + 1) * TP]

        def tile_body(t):
            # int8 -> bf16 (exact integer) conversions
            egob = sp.tile([128, 2, TP], BF, tag="egob")
            for c in range(2):
                nc.scalar.copy(egob[:, c, :], aslot(c, t))
            demb = sp.tile([4, TP], BF, tag="demb")
            nc.scalar.copy(demb, dem8[:, t, :])
            colb = []
            for n in range(NCOL):
                cn = iop.tile([128, 2, TP], BF, tag=f"colb{n}")
                for c in range(2):
                    nc.gpsimd.tensor_copy(cn[:, c, :], aslot(2 + 2 * n + c, t))
                colb.append(cn)

            # ---- demand encoder hidden (b_d1 rides demand row 3) ----
            h_ps = pz.tile([HID, TP], F32, tag="z")
            nc.tensor.matmul(out=h_ps, lhsT=wts[0:4, WD1:WD1 + 128],
                             rhs=demb, start=True, stop=True)
            h_sb = sp.tile([HID, TP], BF, tag="h")
            nc.scalar.activation(out=h_sb, in_=h_ps, func=AF.Relu)

            # ---- q projection (scaled); enc folded in via wqd2T ----
            q_ps = pm.tile([128, 2, TP], F32, tag="m")
            for c in range(2):
                nc.tensor.matmul(out=q_ps[:, c, :], lhsT=wslice(WQ, 0, c),
                                 rhs=egob[:, 0, :], start=True, stop=False)
                nc.tensor.matmul(out=q_ps[:, c, :], lhsT=wslice(WQ, 1, c),
                                 rhs=egob[:, 1, :], start=False, stop=False)
                nc.tensor.matmul(out=q_ps[:, c, :],
                                 lhsT=wts[:, WQD2 + c * 128:WQD2 + c * 128 + 128],
                                 rhs=h_sb, start=False, stop=True)
            q_sb = sp.tile([128, 2, TP], BF, tag="q")
            if has_bias:
                for c in range(2):
                    nc.scalar.activation(out=q_sb[:, c, :], in_=q_ps[:, c, :],
                                         func=AF.Identity,
                                         bias=biases[:, c:c + 1])
            else:
                nc.scalar.activation(out=q_sb, in_=q_ps, func=AF.Copy)

            # ---- k projections + scores ----
            s_ps = pm.tile([128, 2, TP], F32, tag="m")

            def kproj(n):
                k_ps = pkv.tile([128, 2, TP], F32, tag="kv")
                for c in range(2):
                    nc.tensor.matmul(out=k_ps[:, c, :], lhsT=wslice(WK, 0, c),
                                     rhs=colb[n][:, 0, :], start=True, stop=False)
                    nc.tensor.matmul(out=k_ps[:, c, :], lhsT=wslice(WK, 1, c),
                                     rhs=colb[n][:, 1, :], start=False, stop=True)
                return k_ps

            def score(n, k_ps):
                t_sb = sp.tile([128, 2, TP], BF, tag="t")
                nc.vector.tensor_mul(t_sb, q_sb, k_ps)
                nc.tensor.matmul(out=s_ps[32 * n:32 * n + 32, :, :],
                                 lhsT=wts[:, SM:SM + 32], rhs=t_sb,
                                 start=True, stop=True,
                                 tile_position=(0, 32 * n))

            kq = [kproj(0), kproj(1), kproj(2)]
            for n in range(NCOL):
                score(n, kq[n % 3])
                if n + 3 < NCOL:
                    kq[n % 3] = kproj(n + 3)

            # ---- softmax over n (divide-free); denom in s_ps rows 0:4 ----
            e_sb = sp.tile([128, 2, TP], BF, tag="e")
            nc.scalar.activation(out=e_sb, in_=s_ps, func=AF.Exp)
            s_sb = sp.tile([128, 2, TP], BF, tag="s")
            nc.scalar.activation(out=s_sb, in_=s_ps, func=AF.Copy)
            nc.tensor.matmul(out=s_ps[0:4, :, :], lhsT=wts[:, DM:DM + 4],
                             rhs=e_sb, start=True, stop=True)
            nc.scalar.activation(out=s_sb[0:4, :, :], in_=s_ps[0:4, :, :],
                                 func=AF.Ln)

            # ---- attention weights + weighted combine ----
            w_sb = []
            for n in range(NCOL):
                z_ps = pz.tile([128, 2, TP], F32, tag="z")
                nc.tensor.matmul(out=z_ps,
                                 lhsT=wts[:, ZM + n * 128:ZM + n * 128 + 128],
                                 rhs=s_sb, start=True, stop=True)
                a_sb = sp.tile([128, 2, TP], BF, tag="a")
                nc.scalar.activation(out=a_sb, in_=z_ps, func=AF.Exp)
                v_ps = pkv.tile([128, 2, TP], F32, tag="kv")
                for c in range(2):
                    nc.tensor.matmul(out=v_ps[:, c, :], lhsT=wslice(WV, 0, c),
                                     rhs=colb[n][:, 0, :], start=True, stop=False)
                    nc.tensor.matmul(out=v_ps[:, c, :], lhsT=wslice(WV, 1, c),
                                     rhs=colb[n][:, 1, :], start=False, stop=True)
                w_n = wvp.tile([128, 2, TP], BF, tag=f"w{n}")
                nc.vector.tensor_mul(w_n, a_sb, v_ps)
                w_sb.append(w_n)
            u01 = sp.tile([128, 2, TP], BF, tag="u01")
            nc.vector.tensor_add(u01, w_sb[0], w_sb[1])
            u23 = sp.tile([128, 2, TP], BF, tag="u23")
            nc.vector.tensor_add(u23, w_sb[2], w_sb[3])
            u = sp.tile([128, 2, TP], BF, tag="u")
            nc.vector.tensor_add(u, u01, u23)

            # ---- output projection -> resident bf16 buffer ----
            o_ps = pm.tile([128, 2, TP], F32, tag="m")
            for c in range(2):
                nc.tensor.matmul(out=o_ps[:, c, :], lhsT=wslice(WO, 0, c),
                                 rhs=u[:, 0, :], start=True, stop=False)
                nc.tensor.matmul(out=o_ps[:, c, :], lhsT=wslice(WO, 1, c),
                                 rhs=u[:, 1, :], start=False, stop=True)
            px = ts(t, TP)
            if has_bias:
                for c in range(2):
                    nc.scalar.activation(out=outs[:, c, px],
                                         in_=o_ps[:, c, :], func=AF.Identity,
                                         bias=biases[:, 2 + c:3 + c])
            else:
                nc.scalar.activation(out=outs[:, :, px], in_=o_ps,
                                     func=AF.Copy)

        for t in range(NT):
            tile_body(t)

        # ---- int8 output quantization with device-computed scales ----
        nc.vector.tensor_reduce(out=mfin, in_=outs, axis=mybir.AxisListType.X,
                                op=mybir.AluOpType.max,
                                apply_absolute_value=True)
        nc.vector.tensor_scalar_max(mfin, mfin, 1e-30)
        nc.vector.reciprocal(rr, mfin)
        nc.vector.tensor_scalar_mul(rr, rr, 127.0)
        for c in range(2):
            nc.vector.tensor_scalar_mul(qbuf[:, c], outs[:, c], rr[:, c:c + 1])
            nc.sync.dma_start(out=out_d[c, :, 0:HSL * TP], in_=qbuf[:, c])
            nc.sync.dma_start(out=out_d[c, :, HSL * TP:HSL * TP + 4],
                              in_=rr[:, c:c + 1].bitcast(I8))

    if not nc.is_finalized():
        nc.finalize()
    return nc


_PROGRAMS: dict[bool, bass.Bass] = {}


def _get_program(has_bias: bool = False) -> bass.Bass:
    if has_bias not in _PROGRAMS:
        _PROGRAMS[has_bias] = _build_program(has_bias)
    return _PROGRAMS[has_bias]


def _bf16(x):
    return np.asarray(x, dtype=np.float32).astype(ml_dtypes.bfloat16)


_TP_POOL = None


def _pool():
    global _TP_POOL
    if _TP_POOL is None:
        _TP_POOL = ThreadPoolExecutor(8)
    return _TP_POOL


def _chan_quant(x):
    """x: [C, ...] fp32 -> (int8 array, per-channel scale [C])."""
    flat = x.reshape(x.shape[0], -1)
    s = np.abs(flat).max(axis=1) / 127.0
    s[s == 0.0] = 1.0
    inv = (1.0 / s).astype(np.float32)
    y = flat * inv[:, None]
    np.rint(y, out=y)
    q = y.astype(np.int8).reshape(x.shape)
    return q, s


def _quant_acts(ego, pos, col):
    """Quantize ego(+pos) [C,H,W] and collab [N,C,H,W] straight into the
    global concat-ready int8 layout acts[NCORES, 128, ABYTES] (slot 0..1 =
    ego chunks, 2+2n+c = collab n chunk c; the packed weights ride behind
    as raw bytes).  Returns (acts, se[C], sc[C])."""
    acts = np.empty((NCORES, 128, ABYTES), np.int8)
    avs = acts[:, :, 0:WOFF].reshape(NCORES, 128, 10, HSL, W)
    eblocks = [(b, b + 64) for b in range(0, C, 64)]
    emaxs = list(_pool().map(
        lambda blk: np.abs(ego[blk[0]:blk[1]] + pos[blk[0]:blk[1]])
        .max(axis=(1, 2)), eblocks))
    se = np.concatenate(emaxs) / 127.0
    se[se == 0.0] = 1.0
    cmaxs = list(_pool().map(lambda n: np.abs(col[n]).max(axis=(1, 2)),
                             range(NCOL)))
    sc = np.maximum.reduce(cmaxs) / 127.0
    sc[sc == 0.0] = 1.0
    einv = (1.0 / se).astype(np.float32)[:, None, None]
    cinv = (1.0 / sc).astype(np.float32)[:, None, None]

    # Quantize in ~1 MiB slabs through a preallocated temp so the
    # multiply/round/cast passes stay in cache (host may be single-core).
    HB = 4
    buf = np.empty((C, HB, W), np.float32)

    def ework(i):
        for h in range(0, HSL, HB):
            sl = slice(i * HSL + h, i * HSL + h + HB)
            np.add(ego[:, sl], pos[:, sl], out=buf)
            np.multiply(buf, einv, out=buf)
            np.rint(buf, out=buf)
            avs[i, :, 0, h:h + HB] = buf[0:128]
            avs[i, :, 1, h:h + HB] = buf[128:256]
    for i in range(NCORES):
        ework(i)

    def cwork(n, i):
        for h in range(0, HSL, HB):
            sl = slice(i * HSL + h, i * HSL + h + HB)
            np.multiply(col[n, :, sl], cinv, out=buf)
            np.rint(buf, out=buf)
            avs[i, :, 2 + 2 * n, h:h + HB] = buf[0:128]
            avs[i, :, 3 + 2 * n, h:h + HB] = buf[128:256]
    for n in range(NCOL):
        for i in range(NCORES):
            cwork(n, i)
    return acts, se, sc


def _make_masks():
    # Scores for collab n, chunk-local head h live at PSUM/SBUF row 32n+4+h;
    # rows 0..3 of the score tile are later overwritten with L = ln(denom),
    # rows 32n+{0..3,8..31} stay exact zeros.
    smask = np.zeros((128, 32), np.float32)
    for h in range(4):
        smask[32 * h:32 * h + 32, 4 + h] = 1.0
    dmask = np.zeros((128, 4), np.float32)
    for n in range(NCOL):
        for h in range(4):
            dmask[32 * n + 4 + h, h] = 1.0
    zmask = np.zeros((NCOL, 128, 128), np.float32)
    for n in range(NCOL):
        for h in range(4):
            zmask[n, 32 * n + 4 + h, 32 * h:32 * h + 32] = 1.0
            zmask[n, h, 32 * h:32 * h + 32] -= 1.0
    return smask, dmask, zmask


def _pack_wts(wq_s, wk, wv, wo, wqd2, wd1_eff, se, sc):
    """Assemble the merged bf16 weight/mask tensor [128, NW]."""
    wts = np.zeros((128, NW), np.float32)

    def put(base, mat):  # mat [K, M] with K<=128, M=256 -> two 128-col blocks
        wts[:mat.shape[0], base:base + mat.shape[1]] = mat

    put(WQ, (wq_s * se[None, :]).T.reshape(2, 128, C).transpose(1, 0, 2)
        .reshape(128, 512))
    put(WK, (wk * sc[None, :]).T.reshape(2, 128, C).transpose(1, 0, 2)
        .reshape(128, 512))
    put(WV, (wv * sc[None, :]).T.reshape(2, 128, C).transpose(1, 0, 2)
        .reshape(128, 512))
    put(WO, wo.T.reshape(2, 128, C).transpose(1, 0, 2).reshape(128, 512))
    put(WQD2, wqd2.T)                      # [HID, C]
    smask, dmask, zmask = _make_masks()
    for n in range(NCOL):
        put(ZM + n * 128, zmask[n])
    put(WD1, wd1_eff)                      # [4, HID]
    put(SM, smask)
    put(DM, dmask)
    return _bf16(wts)


def kernel(ego_features, ego_demand, collaborator_features,
           w_d1, b_d1, w_d2, b_d2, wq, bq, wk, bk, wv, bv, wo, bo,
           pos_emb):
    ego_features = np.asarray(ego_features, np.float32)
    ego_demand = np.asarray(ego_demand, np.float32)
    collaborator_features = np.asarray(collaborator_features, np.float32)
    w_d1 = np.asarray(w_d1, np.float32); b_d1 = np.asarray(b_d1, np.float32)
    w_d2 = np.asarray(w_d2, np.float32); b_d2 = np.asarray(b_d2, np.float32)
    wq = np.asarray(wq, np.float32); bq = np.asarray(bq, np.float32)
    wk = np.asarray(wk, np.float32); bk = np.asarray(bk, np.float32)
    wv = np.asarray(wv, np.float32); bv = np.asarray(bv, np.float32)
    wo = np.asarray(wo, np.float32); bo = np.asarray(bo, np.float32)
    pos_emb = np.asarray(pos_emb, np.float32)

    # ---- quantize activations (per-channel linear int8) ----
    acts, se, sc = _quant_acts(ego_features[0], pos_emb[0],
                               collaborator_features)
    d8, sd = _chan_quant(ego_demand[0])                 # [3, H, W]

    # ---- fold scales + demand bias into weights ----
    scale = 1.0 / math.sqrt(HD)
    wq_s = wq * scale
    wqd2 = wq_s @ w_d2                                  # [C, HID]
    wd1_eff = np.zeros((4, HID), np.float32)
    wd1_eff[0:3] = (w_d1 * sd[None, :]).T
    wd1_eff[3] = b_d1 / 127.0
    wts = _pack_wts(wq_s, wk, wv, wo, wqd2, wd1_eff, se, sc)
    acts[:, :, WOFF:] = wts.view(np.int8)

    bq_eff = (bq + wq @ b_d2) * scale
    bo_eff = bo + wo @ bv
    has_bias = bool(np.any(bq_eff) or np.any(bo_eff))
    nc = _get_program(has_bias)
    if has_bias:
        bias_np = np.empty((128, 4), np.float32)
        bias_np[:, 0] = bq_eff[0:128]
        bias_np[:, 1] = bq_eff[128:256]
        bias_np[:, 2] = bo_eff[0:128]
        bias_np[:, 3] = bo_eff[128:256]

    d8f = np.empty((4, H, W), np.int8)
    d8f[0:3] = d8
    d8f[3] = 127
    in_maps = []
    for i in range(NCORES):
        sl = slice(i * HSL, (i + 1) * HSL)
        m = {"acts": acts[i], "dem": d8f[:, sl, :]}
        if has_bias:
            m["bias"] = bias_np
        in_maps.append(m)

    res = run_bass_kernel_spmd(nc, in_maps, list(range(NCORES)))

    # ---- dequantize + assemble ----
    out = np.empty((1, C, H, W), np.float32)
    for i in range(NCORES):
        oc = res.results[i]["out"]              # [2, 128, HSL*TP + 4] int8
        rrv = np.ascontiguousarray(oc[:, :, HSL * TP:]).view(np.float32)
        inv = (1.0 / rrv.reshape(C, 1, 1)).astype(np.float32)
        out[0, :, i * HSL:(i + 1) * HSL, :] = \
            oc[:, :, 0:HSL * TP].reshape(C, HSL, TP).astype(np.float32) * inv
    return out


# revision 40
# speedup vs baseline: 1.0885x; 1.0574x over previous
"""Trainium2 Bass kernel for DemandAwareCrossAttention.

Reference computation (per pixel, fully pointwise in (H, W)):
    enc  = w_d2 @ relu(w_d1 @ demand + b_d1) + b_d2
    qs   = ego + enc + pos
    q    = (wq @ qs + bq)   reshaped [8 heads, 32]
    k_n  = wk @ collab_n + bk ; v_n = wv @ collab_n + bv     (n = 0..3)
    s_nm = q_m . k_nm / sqrt(32)
    a    = softmax_n(s)
    u    = sum_n a_nm * v_n            -> [256]
    out  = wo @ u + bo

End-to-end wall time is dominated by the axon tunnel (host<->device
bytes) and per-array dispatch overhead, not device compute.  So:

  - Shard H across the 8 cores (16 rows each); per-core input slices are
    cheap contiguous-chunk views of the full arrays.
  - Activations ship as int8 with per-channel linear scales folded into
    the weights on the host, so the device works on exact small integers
    in bf16; collab is 32 MiB instead of 64, ego 8 instead of 16.
  - All weights + masks pack into ONE bf16 [128, 2980] tensor (fewer
    tunnel round-trips), biases ride a const-127 row of the demand tile.
  - The output returns as int8 plus a per-(channel, chunk) scale that the
    device computes itself (abs-max reduce -> reciprocal); the host
    divides by the returned scale, so reciprocal precision is harmless.

Device layout per tile (one H row, 256 px): channels on partitions,
chunks c in {0,1} of 128.  1x1 convs are PE matmuls (bf16 operands,
fp32 PSUM).  Scores: DVE q*k then a masked matmul sums d within each
head; softmax over n is divide-free (exp, masked-matmul denominator,
ln, masked-matmul broadcast-subtract, exp).  All inputs stay resident
in SBUF; the only DMAs are 12 loads up front and 3 stores at the end.
"""

import math
import numpy as np
import ml_dtypes
from concurrent.futures import ThreadPoolExecutor
from contextlib import ExitStack

import jax

try:
    # Each kernel() call re-jits a fresh wrapper around the bass custom call;
    # the persistent cache turns the ~0.19 s XLA re-compile into a ~10 ms
    # disk hit (the NEFF itself is cached separately by neuronxcc).
    jax.config.update("jax_compilation_cache_dir", "/tmp/jax_cache")
    jax.config.update("jax_persistent_cache_min_compile_time_secs", 0)
    jax.config.update("jax_persistent_cache_min_entry_size_bytes", 0)
except Exception:
    pass

import concourse.bass as bass
import concourse.tile as tile
from concourse import bacc, mybir
from concourse.bass import ts
from concourse.bass_utils import run_bass_kernel_spmd

BF = mybir.dt.bfloat16
F32 = mybir.dt.float32
I8 = mybir.dt.int8
AF = mybir.ActivationFunctionType

# All ScalarE functions used here (Exp/Ln/Relu/Identity/Copy) coexist in the
# "natural_log_exp_and_others" table set, but the table-load pass maps each
# func to the FIRST set containing it, forcing a ~2.7us table switch twice
# per tile.  Shrink the other sets' advertised membership so every func
# resolves to the one shared set -> a single load.
_ACT_FUNCS = {AF.Exp, AF.Ln, AF.Relu, AF.Identity, AF.Copy, AF.Square}
_ORIG_GAT = bacc.get_activation_tables


def _patched_gat(arch):
    tables = _ORIG_GAT(arch)
    return {
        name: (funcs if name == "natural_log_exp_and_others"
               else funcs - _ACT_FUNCS)
        for name, funcs in tables.items()
    }


bacc.get_activation_tables = _patched_gat

C = 256          # model dim
HID = 128        # demand-encoder hidden
NH = 8           # heads
HD = 32          # head dim
NCOL = 4         # collaborators
H, W = 128, 256
NCORES = 8
HSL = H // NCORES          # 16 rows of H per core
TP = W                     # pixels per tile = one H row
NT = HSL                   # 16 tiles

# column offsets inside the packed weight tensor
WQ, WK, WV, WO = 0, 512, 1024, 1536
WQD2 = 2048
ZM = 2304
WD1 = 2816
SM = 2944
DM = 2976
NW = 2980
SLOT = HSL * TP                # 4096 bytes per activation slot
WOFF = 10 * SLOT               # wts bytes live after the 10 activation slots
ABYTES = WOFF + NW * 2


def _build_program(has_bias: bool) -> bass.Bass:
    nc = bacc.Bacc("TRN2", target_bir_lowering=False, debug=False)

    # acts byte layout per partition: slot s*4096.. = activations (s = 0..1
    # ego chunks, 2+2n+c = collab n chunk c), then the packed bf16
    # weight/mask tensor as raw bytes (bitcast back to bf16 on device)
    acts_d = nc.dram_tensor("acts", [128, ABYTES], I8, kind="ExternalInput")
    dem_d = nc.dram_tensor("dem", [4, HSL, TP], I8, kind="ExternalInput")
    if has_bias:
        bias_d = nc.dram_tensor("bias", [128, 4], F32, kind="ExternalInput")
    # per (chunk, channel): 4096 int8 pixels + the 4 raw bytes of the fp32
    # quantization multiplier (bitcast), so the scale rides the same fetch
    out_d = nc.dram_tensor("out", [2, 128, HSL * TP + 4], I8,
                           kind="ExternalOutput")

    with ExitStack() as ctx:
        tc = ctx.enter_context(tile.TileContext(nc))

        wp = ctx.enter_context(tc.tile_pool(name="wp", bufs=1))
        iop = ctx.enter_context(tc.tile_pool(name="io", bufs=2))
        sp = ctx.enter_context(tc.tile_pool(name="sb", bufs=3))
        wvp = ctx.enter_context(tc.tile_pool(name="wv", bufs=2))
        pm = ctx.enter_context(tc.tile_pool(name="pm", bufs=3, space="PSUM"))
        pz = ctx.enter_context(tc.tile_pool(name="pz", bufs=2, space="PSUM"))
        pkv = ctx.enter_context(tc.tile_pool(name="pkv", bufs=3, space="PSUM"))
        # PSUM bank budget: pm{q,s,o}=3 + pz{h,z}=2 + pkv{k,v}=3 = 8

        # ---- resident loads ----
        acts8 = wp.tile([128, ABYTES], I8, tag="acts8")
        nc.sync.dma_start(out=acts8, in_=acts_d[:])
        wts = acts8[:, WOFF:ABYTES].bitcast(BF)      # [128, NW]
        dem8 = wp.tile([4, HSL, TP], I8, tag="dem8")
        nc.sync.dma_start(out=dem8, in_=dem_d[:])
        if has_bias:
            biases = wp.tile([128, 4], F32, tag="bias")
            nc.sync.dma_start(out=biases, in_=bias_d[:])

        outs = wp.tile([128, 2, NT * TP], BF, tag="outs")
        qbuf = wp.tile([128, 2, NT * TP], I8, tag="qbuf")
        mfin = wp.tile([128, 2], F32, tag="mfin")
        rr = wp.tile([128, 2], F32, tag="rr")

        def wslice(base, kc, c):
            off = base + kc * 256 + c * 128
            return wts[:, off:off + 128]

        def aslot(s, t):
            return acts8[:, s * SLOT + t * TP:s * SLOT + (t + 1) * TP]

        def tile_body(t):
            # int8 -> bf16 (exact integer) conversions
            egob = sp.tile([128, 2, TP], BF, tag="egob")
            for c in range(2):
                nc.scalar.copy(egob[:, c, :], aslot(c, t))
            demb = sp.tile([4, TP], BF, tag="demb")
            nc.scalar.copy(demb, dem8[:, t, :])
            colb = []
            for n in range(NCOL):
                cn = iop.tile([128, 2, TP], BF, tag=f"colb{n}")
                for c in range(2):
                    nc.gpsimd.tensor_copy(cn[:, c, :], aslot(2 + 2 * n + c, t))
                colb.append(cn)

            # ---- demand encoder hidden (b_d1 rides demand row 3) ----
            h_ps = pz.tile([HID, TP], F32, tag="z")
            nc.tensor.matmul(out=h_ps, lhsT=wts[0:4, WD1:WD1 + 128],
                             rhs=demb, start=True, stop=True)
            h_sb = sp.tile([HID, TP], BF, tag="h")
            nc.scalar.activation(out=h_sb, in_=h_ps, func=AF.Relu)

            # ---- q projection (scaled); enc folded in via wqd2T ----
            q_ps = pm.tile([128, 2, TP], F32, tag="m")
            for c in range(2):
                nc.tensor.matmul(out=q_ps[:, c, :], lhsT=wslice(WQ, 0, c),
                                 rhs=egob[:, 0, :], start=True, stop=False)
                nc.tensor.matmul(out=q_ps[:, c, :], lhsT=wslice(WQ, 1, c),
                                 rhs=egob[:, 1, :], start=False, stop=False)
                nc.tensor.matmul(out=q_ps[:, c, :],
                                 lhsT=wts[:, WQD2 + c * 128:WQD2 + c * 128 + 128],
                                 rhs=h_sb, start=False, stop=True)
            q_sb = sp.tile([128, 2, TP], BF, tag="q")
            if has_bias:
                for c in range(2):
                    nc.scalar.activation(out=q_sb[:, c, :], in_=q_ps[:, c, :],
                                         func=AF.Identity,
                                         bias=biases[:, c:c + 1])
            else:
                nc.scalar.activation(out=q_sb, in_=q_ps, func=AF.Copy)

            # ---- k projections + scores ----
            s_ps = pm.tile([128, 2, TP], F32, tag="m")

            def kproj(n):
                k_ps = pkv.tile([128, 2, TP], F32, tag="kv")
                for c in range(2):
                    nc.tensor.matmul(out=k_ps[:, c, :], lhsT=wslice(WK, 0, c),
                                     rhs=colb[n][:, 0, :], start=True, stop=False)
                    nc.tensor.matmul(out=k_ps[:, c, :], lhsT=wslice(WK, 1, c),
                                     rhs=colb[n][:, 1, :], start=False, stop=True)
                return k_ps

            def score(n, k_ps):
                t_sb = sp.tile([128, 2, TP], BF, tag="t")
                nc.vector.tensor_mul(t_sb, q_sb, k_ps)
                nc.tensor.matmul(out=s_ps[32 * n:32 * n + 32, :, :],
                                 lhsT=wts[:, SM:SM + 32], rhs=t_sb,
                                 start=True, stop=True,
                                 tile_position=(0, 32 * n))

            kq = [kproj(0), kproj(1), kproj(2)]
            for n in range(NCOL):
                score(n, kq[n % 3])
                if n + 3 < NCOL:
                    kq[n % 3] = kproj(n + 3)

            # ---- softmax over n (divide-free); denom in s_ps rows 0:4 ----
            e_sb = sp.tile([128, 2, TP], BF, tag="e")
            nc.scalar.activation(out=e_sb, in_=s_ps, func=AF.Exp)
            s_sb = sp.tile([128, 2, TP], BF, tag="s")
            nc.scalar.activation(out=s_sb, in_=s_ps, func=AF.Copy)
            nc.tensor.matmul(out=s_ps[0:4, :, :], lhsT=wts[:, DM:DM + 4],
                             rhs=e_sb, start=True, stop=True)
            nc.scalar.activation(out=s_sb[0:4, :, :], in_=s_ps[0:4, :, :],
                                 func=AF.Ln)

            # ---- attention weights + weighted combine ----
            w_sb = []
            for n in range(NCOL):
                z_ps = pz.tile([128, 2, TP], F32, tag="z")
                nc.tensor.matmul(out=z_ps,
                                 lhsT=wts[:, ZM + n * 128:ZM + n * 128 + 128],
                                 rhs=s_sb, start=True, stop=True)
                a_sb = sp.tile([128, 2, TP], BF, tag="a")
                nc.scalar.activation(out=a_sb, in_=z_ps, func=AF.Exp)
                v_ps = pkv.tile([128, 2, TP], F32, tag="kv")
                for c in range(2):
                    nc.tensor.matmul(out=v_ps[:, c, :], lhsT=wslice(WV, 0, c),
                                     rhs=colb[n][:, 0, :], start=True, stop=False)
                    nc.tensor.matmul(out=v_ps[:, c, :], lhsT=wslice(WV, 1, c),
                                     rhs=colb[n][:, 1, :], start=False, stop=True)
                w_n = wvp.tile([128, 2, TP], BF, tag=f"w{n}")
                nc.vector.tensor_mul(w_n, a_sb, v_ps)
                w_sb.append(w_n)
            u01 = sp.tile([128, 2, TP], BF, tag="u01")
            nc.vector.tensor_add(u01, w_sb[0], w_sb[1])
            u23 = sp.tile([128, 2, TP], BF, tag="u23")
            nc.vector.tensor_add(u23, w_sb[2], w_sb[3])
            u = sp.tile([128, 2, TP], BF, tag="u")
            nc.vector.tensor_add(u, u01, u23)

            # ---- output projection -> resident bf16 buffer ----
            o_ps = pm.tile([128, 2, TP], F32, tag="m")
            for c in range(2):
                nc.tensor.matmul(out=o_ps[:, c, :], lhsT=wslice(WO, 0, c),
                                 rhs=u[:, 0, :], start=True, stop=False)
                nc.tensor.matmul(out=o_ps[:, c, :], lhsT=wslice(WO, 1, c),
                                 rhs=u[:, 1, :], start=False, stop=True)
            px = ts(t, TP)
            if has_bias:
                for c in range(2):
                    nc.scalar.activation(out=outs[:, c, px],
                                         in_=o_ps[:, c, :], func=AF.Identity,
                                         bias=biases[:, 2 + c:3 + c])
            else:
                nc.scalar.activation(out=outs[:, :, px], in_=o_ps,
                                     func=AF.Copy)

        for t in range(NT):
            tile_body(t)

        # ---- int8 output quantization with device-computed scales ----
        nc.vector.tensor_reduce(out=mfin, in_=outs, axis=mybir.AxisListType.X,
                                op=mybir.AluOpType.max,
                                apply_absolute_value=True)
        nc.vector.tensor_scalar_max(mfin, mfin, 1e-30)
        nc.vector.reciprocal(rr, mfin)
        nc.vector.tensor_scalar_mul(rr, rr, 127.0)
        for c in range(2):
            nc.vector.tensor_scalar_mul(qbuf[:, c], outs[:, c], rr[:, c:c + 1])
            nc.sync.dma_start(out=out_d[c, :, 0:HSL * TP], in_=qbuf[:, c])
            nc.sync.dma_start(out=out_d[c, :, HSL * TP:HSL * TP + 4],
                              in_=rr[:, c:c + 1].bitcast(I8))

    if not nc.is_finalized():
        nc.finalize()
    return nc


_PROGRAMS: dict[bool, bass.Bass] = {}


def _get_program(has_bias: bool = False) -> bass.Bass:
    if has_bias not in _PROGRAMS:
        _PROGRAMS[has_bias] = _build_program(has_bias)
    return _PROGRAMS[has_bias]


def _bf16(x):
    return np.asarray(x, dtype=np.float32).astype(ml_dtypes.bfloat16)


_TP_POOL = None


def _pool():
    global _TP_POOL
    if _TP_POOL is None:
        _TP_POOL = ThreadPoolExecutor(8)
    return _TP_POOL


def _chan_quant(x):
    """x: [C, ...] fp32 -> (int8 array, per-channel scale [C])."""
    flat = x.reshape(x.shape[0], -1)
    s = np.abs(flat).max(axis=1) / 127.0
    s[s == 0.0] = 1.0
    inv = (1.0 / s).astype(np.float32)
    y = flat * inv[:, None]
    np.rint(y, out=y)
    q = y.astype(np.int8).reshape(x.shape)
    return q, s


def _quant_acts(ego, pos, col):
    """Quantize ego(+pos) [C,H,W] and collab [N,C,H,W] straight into the
    global concat-ready int8 layout acts[NCORES, 128, ABYTES] (slot 0..1 =
    ego chunks, 2+2n+c = collab n chunk c; the packed weights ride behind
    as raw bytes).  Returns (acts, se[C], sc[C])."""
    acts = np.empty((NCORES, 128, ABYTES), np.int8)
    avs = acts[:, :, 0:WOFF].reshape(NCORES, 128, 10, HSL, W)
    # min/max reduction passes avoid the 33 MiB np.abs temporaries; for ego
    # the sum-of-extrema is an upper bound on absmax(ego+pos) — exact when
    # pos == 0, merely a slightly conservative scale otherwise
    se = np.maximum(ego.max(axis=(1, 2)) + pos.max(axis=(1, 2)),
                    -(ego.min(axis=(1, 2)) + pos.min(axis=(1, 2)))) / 127.0
    cmx = col.max(axis=(0, 2, 3))
    cmn = col.min(axis=(0, 2, 3))
    sc = np.maximum(cmx, -cmn) / 127.0
    se[se <= 0.0] = 1.0
    sc[sc == 0.0] = 1.0
    einv = (1.0 / se).astype(np.float32)[:, None, None]
    cinv = (1.0 / sc).astype(np.float32)[:, None, None]

    # Quantize in ~1 MiB slabs through a preallocated temp so the
    # multiply/round/cast passes stay in cache (host may be single-core).
    HB = 4
    buf = np.empty((C, HB, W), np.float32)

    def ework(i):
        for h in range(0, HSL, HB):
            sl = slice(i * HSL + h, i * HSL + h + HB)
            np.add(ego[:, sl], pos[:, sl], out=buf)
            np.multiply(buf, einv, out=buf)
            np.rint(buf, out=buf)
            avs[i, :, 0, h:h + HB] = buf[0:128]
            avs[i, :, 1, h:h + HB] = buf[128:256]
    for i in range(NCORES):
        ework(i)

    def cwork(n, i):
        for h in range(0, HSL, HB):
            sl = slice(i * HSL + h, i * HSL + h + HB)
            np.multiply(col[n, :, sl], cinv, out=buf)
            np.rint(buf, out=buf)
            avs[i, :, 2 + 2 * n, h:h + HB] = buf[0:128]
            avs[i, :, 3 + 2 * n, h:h + HB] = buf[128:256]
    for n in range(NCOL):
        for i in range(NCORES):
            cwork(n, i)
    return acts, se, sc


def _make_masks():
    # Scores for collab n, chunk-local head h live at PSUM/SBUF row 32n+4+h;
    # rows 0..3 of the score tile are later overwritten with L = ln(denom),
    # rows 32n+{0..3,8..31} stay exact zeros.
    smask = np.zeros((128, 32), np.float32)
    for h in range(4):
        smask[32 * h:32 * h + 32, 4 + h] = 1.0
    dmask = np.zeros((128, 4), np.float32)
    for n in range(NCOL):
        for h in range(4):
            dmask[32 * n + 4 + h, h] = 1.0
    zmask = np.zeros((NCOL, 128, 128), np.float32)
    for n in range(NCOL):
        for h in range(4):
            zmask[n, 32 * n + 4 + h, 32 * h:32 * h + 32] = 1.0
            zmask[n, h, 32 * h:32 * h + 32] -= 1.0
    return smask, dmask, zmask


def _pack_wts(wq_s, wk, wv, wo, wqd2, wd1_eff, se, sc):
    """Assemble the merged bf16 weight/mask tensor [128, NW]."""
    wts = np.zeros((128, NW), np.float32)

    def put(base, mat):  # mat [K, M] with K<=128, M=256 -> two 128-col blocks
        wts[:mat.shape[0], base:base + mat.shape[1]] = mat

    put(WQ, (wq_s * se[None, :]).T.reshape(2, 128, C).transpose(1, 0, 2)
        .reshape(128, 512))
    put(WK, (wk * sc[None, :]).T.reshape(2, 128, C).transpose(1, 0, 2)
        .reshape(128, 512))
    put(WV, (wv * sc[None, :]).T.reshape(2, 128, C).transpose(1, 0, 2)
        .reshape(128, 512))
    put(WO, wo.T.reshape(2, 128, C).transpose(1, 0, 2).reshape(128, 512))
    put(WQD2, wqd2.T)                      # [HID, C]
    smask, dmask, zmask = _make_masks()
    for n in range(NCOL):
        put(ZM + n * 128, zmask[n])
    put(WD1, wd1_eff)                      # [4, HID]
    put(SM, smask)
    put(DM, dmask)
    return _bf16(wts)


def kernel(ego_features, ego_demand, collaborator_features,
           w_d1, b_d1, w_d2, b_d2, wq, bq, wk, bk, wv, bv, wo, bo,
           pos_emb):
    ego_features = np.asarray(ego_features, np.float32)
    ego_demand = np.asarray(ego_demand, np.float32)
    collaborator_features = np.asarray(collaborator_features, np.float32)
    w_d1 = np.asarray(w_d1, np.float32); b_d1 = np.asarray(b_d1, np.float32)
    w_d2 = np.asarray(w_d2, np.float32); b_d2 = np.asarray(b_d2, np.float32)
    wq = np.asarray(wq, np.float32); bq = np.asarray(bq, np.float32)
    wk = np.asarray(wk, np.float32); bk = np.asarray(bk, np.float32)
    wv = np.asarray(wv, np.float32); bv = np.asarray(bv, np.float32)
    wo = np.asarray(wo, np.float32); bo = np.asarray(bo, np.float32)
    pos_emb = np.asarray(pos_emb, np.float32)

    # ---- quantize activations (per-channel linear int8) ----
    acts, se, sc = _quant_acts(ego_features[0], pos_emb[0],
                               collaborator_features)
    d8, sd = _chan_quant(ego_demand[0])                 # [3, H, W]

    # ---- fold scales + demand bias into weights ----
    scale = 1.0 / math.sqrt(HD)
    wq_s = wq * scale
    wqd2 = wq_s @ w_d2                                  # [C, HID]
    wd1_eff = np.zeros((4, HID), np.float32)
    wd1_eff[0:3] = (w_d1 * sd[None, :]).T
    wd1_eff[3] = b_d1 / 127.0
    wts = _pack_wts(wq_s, wk, wv, wo, wqd2, wd1_eff, se, sc)
    acts[:, :, WOFF:] = wts.view(np.int8)

    bq_eff = (bq + wq @ b_d2) * scale
    bo_eff = bo + wo @ bv
    has_bias = bool(np.any(bq_eff) or np.any(bo_eff))
    nc = _get_program(has_bias)
    if has_bias:
        bias_np = np.empty((128, 4), np.float32)
        bias_np[:, 0] = bq_eff[0:128]
        bias_np[:, 1] = bq_eff[128:256]
        bias_np[:, 2] = bo_eff[0:128]
        bias_np[:, 3] = bo_eff[128:256]

    d8f = np.empty((4, H, W), np.int8)
    d8f[0:3] = d8
    d8f[3] = 127
    in_maps = []
    for i in range(NCORES):
        sl = slice(i * HSL, (i + 1) * HSL)
        m = {"acts": acts[i], "dem": d8f[:, sl, :]}
        if has_bias:
            m["bias"] = bias_np
        in_maps.append(m)

    res = run_bass_kernel_spmd(nc, in_maps, list(range(NCORES)))

    # ---- dequantize + assemble ----
    out = np.empty((1, C, H, W), np.float32)
    for i in range(NCORES):
        oc = res.results[i]["out"]              # [2, 128, HSL*TP + 4] int8
        rrv = np.ascontiguousarray(oc[:, :, HSL * TP:]).view(np.float32)
        inv = (1.0 / rrv.reshape(C, 1, 1)).astype(np.float32)
        out[0, :, i * HSL:(i + 1) * HSL, :] = \
            oc[:, :, 0:HSL * TP].reshape(C, HSL, TP).astype(np.float32) * inv
    return out


# revision 42
# speedup vs baseline: 1.0912x; 1.0025x over previous
"""Trainium2 Bass kernel for DemandAwareCrossAttention.

Reference computation (per pixel, fully pointwise in (H, W)):
    enc  = w_d2 @ relu(w_d1 @ demand + b_d1) + b_d2
    qs   = ego + enc + pos
    q    = (wq @ qs + bq)   reshaped [8 heads, 32]
    k_n  = wk @ collab_n + bk ; v_n = wv @ collab_n + bv     (n = 0..3)
    s_nm = q_m . k_nm / sqrt(32)
    a    = softmax_n(s)
    u    = sum_n a_nm * v_n            -> [256]
    out  = wo @ u + bo

End-to-end wall time is dominated by the axon tunnel (host<->device
bytes) and per-array dispatch overhead, not device compute.  So:

  - Shard H across the 8 cores (16 rows each); per-core input slices are
    cheap contiguous-chunk views of the full arrays.
  - Activations ship as int8 with per-channel linear scales folded into
    the weights on the host, so the device works on exact small integers
    in bf16; collab is 32 MiB instead of 64, ego 8 instead of 16.
  - All weights + masks pack into ONE bf16 [128, 2980] tensor (fewer
    tunnel round-trips), biases ride a const-127 row of the demand tile.
  - The output returns as int8 plus a per-(channel, chunk) scale that the
    device computes itself (abs-max reduce -> reciprocal); the host
    divides by the returned scale, so reciprocal precision is harmless.

Device layout per tile (one H row, 256 px): channels on partitions,
chunks c in {0,1} of 128.  1x1 convs are PE matmuls (bf16 operands,
fp32 PSUM).  Scores: DVE q*k then a masked matmul sums d within each
head; softmax over n is divide-free (exp, masked-matmul denominator,
ln, masked-matmul broadcast-subtract, exp).  All inputs stay resident
in SBUF; the only DMAs are 12 loads up front and 3 stores at the end.
"""

import math
import numpy as np
import ml_dtypes
from contextlib import ExitStack

import jax

try:
    # Each kernel() call re-jits a fresh wrapper around the bass custom call;
    # the persistent cache turns the ~0.19 s XLA re-compile into a ~10 ms
    # disk hit (the NEFF itself is cached separately by neuronxcc).
    jax.config.update("jax_compilation_cache_dir", "/tmp/jax_cache")
    jax.config.update("jax_persistent_cache_min_compile_time_secs", 0)
    jax.config.update("jax_persistent_cache_min_entry_size_bytes", 0)
except Exception:
    pass

import concourse.bass as bass
import concourse.tile as tile
from concourse import bacc, mybir
from concourse.bass import ts
from concourse.bass_utils import run_bass_kernel_spmd

BF = mybir.dt.bfloat16
F32 = mybir.dt.float32
I8 = mybir.dt.int8
AF = mybir.ActivationFunctionType

# All ScalarE functions used here (Exp/Ln/Relu/Identity/Copy) coexist in the
# "natural_log_exp_and_others" table set, but the table-load pass maps each
# func to the FIRST set containing it, forcing a ~2.7us table switch twice
# per tile.  Shrink the other sets' advertised membership so every func
# resolves to the one shared set -> a single load.
_ACT_FUNCS = {AF.Exp, AF.Ln, AF.Relu, AF.Identity, AF.Copy, AF.Square}
_ORIG_GAT = bacc.get_activation_tables


def _patched_gat(arch):
    tables = _ORIG_GAT(arch)
    return {
        name: (funcs if name == "natural_log_exp_and_others"
               else funcs - _ACT_FUNCS)
        for name, funcs in tables.items()
    }


bacc.get_activation_tables = _patched_gat

C = 256          # model dim
HID = 128        # demand-encoder hidden
NH = 8           # heads
HD = 32          # head dim
NCOL = 4         # collaborators
H, W = 128, 256
NCORES = 8
HSL = H // NCORES          # 16 rows of H per core
TP = W                     # pixels per tile = one H row
NT = HSL                   # 16 tiles

# column offsets inside the packed weight tensor
WQ, WK, WV, WO = 0, 512, 1024, 1536
WQD2 = 2048
ZM = 2304
WD1 = 2816
SM = 2944
DM = 2976
NW = 2980
SLOT = HSL * TP                # 4096 bytes per activation slot
WOFF = 10 * SLOT               # wts bytes live after the 10 activation slots
ABYTES = WOFF + NW * 2


def _build_program(has_bias: bool) -> bass.Bass:
    nc = bacc.Bacc("TRN2", target_bir_lowering=False, debug=False)

    # acts byte layout per partition: slot s*4096.. = activations (s = 0..1
    # ego chunks, 2+2n+c = collab n chunk c), then the packed bf16
    # weight/mask tensor as raw bytes (bitcast back to bf16 on device)
    acts_d = nc.dram_tensor("acts", [128, ABYTES], I8, kind="ExternalInput")
    dem_d = nc.dram_tensor("dem", [4, HSL, TP], I8, kind="ExternalInput")
    if has_bias:
        bias_d = nc.dram_tensor("bias", [128, 4], F32, kind="ExternalInput")
    # per (chunk, channel): 4096 int8 pixels + the 4 raw bytes of the fp32
    # quantization multiplier (bitcast), so the scale rides the same fetch
    out_d = nc.dram_tensor("out", [2, 128, HSL * TP + 4], I8,
                           kind="ExternalOutput")

    with ExitStack() as ctx:
        tc = ctx.enter_context(tile.TileContext(nc))

        wp = ctx.enter_context(tc.tile_pool(name="wp", bufs=1))
        iop = ctx.enter_context(tc.tile_pool(name="io", bufs=2))
        sp = ctx.enter_context(tc.tile_pool(name="sb", bufs=3))
        wvp = ctx.enter_context(tc.tile_pool(name="wv", bufs=2))
        pm = ctx.enter_context(tc.tile_pool(name="pm", bufs=3, space="PSUM"))
        pz = ctx.enter_context(tc.tile_pool(name="pz", bufs=2, space="PSUM"))
        pkv = ctx.enter_context(tc.tile_pool(name="pkv", bufs=3, space="PSUM"))
        # PSUM bank budget: pm{q,s,o}=3 + pz{h,z}=2 + pkv{k,v}=3 = 8

        # ---- resident loads ----
        acts8 = wp.tile([128, ABYTES], I8, tag="acts8")
        nc.sync.dma_start(out=acts8, in_=acts_d[:])
        wts = acts8[:, WOFF:ABYTES].bitcast(BF)      # [128, NW]
        dem8 = wp.tile([4, HSL, TP], I8, tag="dem8")
        nc.sync.dma_start(out=dem8, in_=dem_d[:])
        if has_bias:
            biases = wp.tile([128, 4], F32, tag="bias")
            nc.sync.dma_start(out=biases, in_=bias_d[:])

        outs = wp.tile([128, 2, NT * TP], BF, tag="outs")
        qbuf = wp.tile([128, 2, NT * TP], I8, tag="qbuf")
        mfin = wp.tile([128, 2], F32, tag="mfin")
        rr = wp.tile([128, 2], F32, tag="rr")

        def wslice(base, kc, c):
            off = base + kc * 256 + c * 128
            return wts[:, off:off + 128]

        def aslot(s, t):
            return acts8[:, s * SLOT + t * TP:s * SLOT + (t + 1) * TP]

        def tile_body(t):
            # int8 -> bf16 (exact integer) conversions
            egob = sp.tile([128, 2, TP], BF, tag="egob")
            for c in range(2):
                nc.scalar.copy(egob[:, c, :], aslot(c, t))
            demb = sp.tile([4, TP], BF, tag="demb")
            nc.scalar.copy(demb, dem8[:, t, :])
            colb = []
            for n in range(NCOL):
                cn = iop.tile([128, 2, TP], BF, tag=f"colb{n}")
                for c in range(2):
                    nc.gpsimd.tensor_copy(cn[:, c, :], aslot(2 + 2 * n + c, t))
                colb.append(cn)

            # ---- demand encoder hidden (b_d1 rides demand row 3) ----
            h_ps = pz.tile([HID, TP], F32, tag="z")
            nc.tensor.matmul(out=h_ps, lhsT=wts[0:4, WD1:WD1 + 128],
                             rhs=demb, start=True, stop=True)
            h_sb = sp.tile([HID, TP], BF, tag="h")
            nc.scalar.activation(out=h_sb, in_=h_ps, func=AF.Relu)

            # ---- q projection (scaled); enc folded in via wqd2T ----
            q_ps = pm.tile([128, 2, TP], F32, tag="m")
            for c in range(2):
                nc.tensor.matmul(out=q_ps[:, c, :], lhsT=wslice(WQ, 0, c),
                                 rhs=egob[:, 0, :], start=True, stop=False)
                nc.tensor.matmul(out=q_ps[:, c, :], lhsT=wslice(WQ, 1, c),
                                 rhs=egob[:, 1, :], start=False, stop=False)
                nc.tensor.matmul(out=q_ps[:, c, :],
                                 lhsT=wts[:, WQD2 + c * 128:WQD2 + c * 128 + 128],
                                 rhs=h_sb, start=False, stop=True)
            q_sb = sp.tile([128, 2, TP], BF, tag="q")
            if has_bias:
                for c in range(2):
                    nc.scalar.activation(out=q_sb[:, c, :], in_=q_ps[:, c, :],
                                         func=AF.Identity,
                                         bias=biases[:, c:c + 1])
            else:
                nc.scalar.activation(out=q_sb, in_=q_ps, func=AF.Copy)

            # ---- k projections + scores ----
            s_ps = pm.tile([128, 2, TP], F32, tag="m")

            def kproj(n):
                k_ps = pkv.tile([128, 2, TP], F32, tag="kv")
                for c in range(2):
                    nc.tensor.matmul(out=k_ps[:, c, :], lhsT=wslice(WK, 0, c),
                                     rhs=colb[n][:, 0, :], start=True, stop=False)
                    nc.tensor.matmul(out=k_ps[:, c, :], lhsT=wslice(WK, 1, c),
                                     rhs=colb[n][:, 1, :], start=False, stop=True)
                return k_ps

            def score(n, k_ps):
                t_sb = sp.tile([128, 2, TP], BF, tag="t")
                nc.vector.tensor_mul(t_sb, q_sb, k_ps)
                nc.tensor.matmul(out=s_ps[32 * n:32 * n + 32, :, :],
                                 lhsT=wts[:, SM:SM + 32], rhs=t_sb,
                                 start=True, stop=True,
                                 tile_position=(0, 32 * n))

            kq = [kproj(0), kproj(1), kproj(2)]
            for n in range(NCOL):
                score(n, kq[n % 3])
                if n + 3 < NCOL:
                    kq[n % 3] = kproj(n + 3)

            # ---- softmax over n (divide-free); denom in s_ps rows 0:4 ----
            e_sb = sp.tile([128, 2, TP], BF, tag="e")
            nc.scalar.activation(out=e_sb, in_=s_ps, func=AF.Exp)
            s_sb = sp.tile([128, 2, TP], BF, tag="s")
            nc.scalar.activation(out=s_sb, in_=s_ps, func=AF.Copy)
            nc.tensor.matmul(out=s_ps[0:4, :, :], lhsT=wts[:, DM:DM + 4],
                             rhs=e_sb, start=True, stop=True)
            nc.scalar.activation(out=s_sb[0:4, :, :], in_=s_ps[0:4, :, :],
                                 func=AF.Ln)

            # ---- attention weights + weighted combine ----
            w_sb = []
            for n in range(NCOL):
                z_ps = pz.tile([128, 2, TP], F32, tag="z")
                nc.tensor.matmul(out=z_ps,
                                 lhsT=wts[:, ZM + n * 128:ZM + n * 128 + 128],
                                 rhs=s_sb, start=True, stop=True)
                a_sb = sp.tile([128, 2, TP], BF, tag="a")
                nc.scalar.activation(out=a_sb, in_=z_ps, func=AF.Exp)
                v_ps = pkv.tile([128, 2, TP], F32, tag="kv")
                for c in range(2):
                    nc.tensor.matmul(out=v_ps[:, c, :], lhsT=wslice(WV, 0, c),
                                     rhs=colb[n][:, 0, :], start=True, stop=False)
                    nc.tensor.matmul(out=v_ps[:, c, :], lhsT=wslice(WV, 1, c),
                                     rhs=colb[n][:, 1, :], start=False, stop=True)
                w_n = wvp.tile([128, 2, TP], BF, tag=f"w{n}")
                nc.vector.tensor_mul(w_n, a_sb, v_ps)
                w_sb.append(w_n)
            u01 = sp.tile([128, 2, TP], BF, tag="u01")
            nc.vector.tensor_add(u01, w_sb[0], w_sb[1])
            u23 = sp.tile([128, 2, TP], BF, tag="u23")
            nc.vector.tensor_add(u23, w_sb[2], w_sb[3])
            u = sp.tile([128, 2, TP], BF, tag="u")
            nc.vector.tensor_add(u, u01, u23)

            # ---- output projection -> resident bf16 buffer ----
            o_ps = pm.tile([128, 2, TP], F32, tag="m")
            for c in range(2):
                nc.tensor.matmul(out=o_ps[:, c, :], lhsT=wslice(WO, 0, c),
                                 rhs=u[:, 0, :], start=True, stop=False)
                nc.tensor.matmul(out=o_ps[:, c, :], lhsT=wslice(WO, 1, c),
                                 rhs=u[:, 1, :], start=False, stop=True)
            px = ts(t, TP)
            if has_bias:
                for c in range(2):
                    nc.scalar.activation(out=outs[:, c, px],
                                         in_=o_ps[:, c, :], func=AF.Identity,
                                         bias=biases[:, 2 + c:3 + c])
            else:
                nc.scalar.activation(out=outs[:, :, px], in_=o_ps,
                                     func=AF.Copy)

        for t in range(NT):
            tile_body(t)

        # ---- int8 output quantization with device-computed scales ----
        nc.vector.tensor_reduce(out=mfin, in_=outs, axis=mybir.AxisListType.X,
                                op=mybir.AluOpType.max,
                                apply_absolute_value=True)
        nc.vector.tensor_scalar_max(mfin, mfin, 1e-30)
        nc.vector.reciprocal(rr, mfin)
        nc.vector.tensor_scalar_mul(rr, rr, 127.0)
        for c in range(2):
            nc.vector.tensor_scalar_mul(qbuf[:, c], outs[:, c], rr[:, c:c + 1])
            nc.sync.dma_start(out=out_d[c, :, 0:HSL * TP], in_=qbuf[:, c])
            nc.sync.dma_start(out=out_d[c, :, HSL * TP:HSL * TP + 4],
                              in_=rr[:, c:c + 1].bitcast(I8))

    if not nc.is_finalized():
        nc.finalize()
    return nc


_PROGRAMS: dict[bool, bass.Bass] = {}


def _get_program(has_bias: bool = False) -> bass.Bass:
    if has_bias not in _PROGRAMS:
        _PROGRAMS[has_bias] = _build_program(has_bias)
    return _PROGRAMS[has_bias]


def _bf16(x):
    return np.asarray(x, dtype=np.float32).astype(ml_dtypes.bfloat16)


def _chan_quant(x):
    """x: [C, ...] fp32 -> (int8 array, per-channel scale [C])."""
    flat = x.reshape(x.shape[0], -1)
    s = np.abs(flat).max(axis=1) / 127.0
    s[s == 0.0] = 1.0
    inv = (1.0 / s).astype(np.float32)
    y = flat * inv[:, None]
    np.rint(y, out=y)
    q = y.astype(np.int8).reshape(x.shape)
    return q, s


def _quant_acts(ego, pos, col):
    """Quantize ego(+pos) [C,H,W] and collab [N,C,H,W] straight into the
    global concat-ready int8 layout acts[NCORES, 128, ABYTES] (slot 0..1 =
    ego chunks, 2+2n+c = collab n chunk c; the packed weights ride behind
    as raw bytes).  Returns (acts, se[C], sc[C])."""
    acts = np.empty((NCORES, 128, ABYTES), np.int8)
    avs = acts[:, :, 0:WOFF].reshape(NCORES, 128, 10, HSL, W)
    # min/max reduction passes avoid the 33 MiB np.abs temporaries; for ego
    # the sum-of-extrema is an upper bound on absmax(ego+pos) — exact when
    # pos == 0, merely a slightly conservative scale otherwise.  Ego only
    # shifts attention logits (softmax-smoothed), so +-31 levels suffice
    # and the narrower byte range compresses better on the wire.
    se = np.maximum(ego.max(axis=(1, 2)) + pos.max(axis=(1, 2)),
                    -(ego.min(axis=(1, 2)) + pos.min(axis=(1, 2)))) / 31.0
    cmx = col.max(axis=(0, 2, 3))
    cmn = col.min(axis=(0, 2, 3))
    sc = np.maximum(cmx, -cmn) / 127.0
    se[se <= 0.0] = 1.0
    sc[sc == 0.0] = 1.0
    einv = (1.0 / se).astype(np.float32)[:, None, None]
    cinv = (1.0 / sc).astype(np.float32)[:, None, None]

    # Quantize in ~1 MiB slabs through a preallocated temp so the
    # multiply/round/cast passes stay in cache (host may be single-core).
    HB = 4
    buf = np.empty((C, HB, W), np.float32)

    def ework(i):
        for h in range(0, HSL, HB):
            sl = slice(i * HSL + h, i * HSL + h + HB)
            np.add(ego[:, sl], pos[:, sl], out=buf)
            np.multiply(buf, einv, out=buf)
            np.rint(buf, out=buf)
            avs[i, :, 0, h:h + HB] = buf[0:128]
            avs[i, :, 1, h:h + HB] = buf[128:256]
    for i in range(NCORES):
        ework(i)

    def cwork(n, i):
        for h in range(0, HSL, HB):
            sl = slice(i * HSL + h, i * HSL + h + HB)
            np.multiply(col[n, :, sl], cinv, out=buf)
            np.rint(buf, out=buf)
            avs[i, :, 2 + 2 * n, h:h + HB] = buf[0:128]
            avs[i, :, 3 + 2 * n, h:h + HB] = buf[128:256]
    for n in range(NCOL):
        for i in range(NCORES):
            cwork(n, i)
    return acts, se, sc


def _make_masks():
    # Scores for collab n, chunk-local head h live at PSUM/SBUF row 32n+4+h;
    # rows 0..3 of the score tile are later overwritten with L = ln(denom),
    # rows 32n+{0..3,8..31} stay exact zeros.
    smask = np.zeros((128, 32), np.float32)
    for h in range(4):
        smask[32 * h:32 * h + 32, 4 + h] = 1.0
    dmask = np.zeros((128, 4), np.float32)
    for n in range(NCOL):
        for h in range(4):
            dmask[32 * n + 4 + h, h] = 1.0
    zmask = np.zeros((NCOL, 128, 128), np.float32)
    for n in range(NCOL):
        for h in range(4):
            zmask[n, 32 * n + 4 + h, 32 * h:32 * h + 32] = 1.0
            zmask[n, h, 32 * h:32 * h + 32] -= 1.0
    return smask, dmask, zmask


def _pack_wts(wq_s, wk, wv, wo, wqd2, wd1_eff, se, sc):
    """Assemble the merged bf16 weight/mask tensor [128, NW]."""
    wts = np.zeros((128, NW), np.float32)

    def put(base, mat):  # mat [K, M] with K<=128, M=256 -> two 128-col blocks
        wts[:mat.shape[0], base:base + mat.shape[1]] = mat

    put(WQ, (wq_s * se[None, :]).T.reshape(2, 128, C).transpose(1, 0, 2)
        .reshape(128, 512))
    put(WK, (wk * sc[None, :]).T.reshape(2, 128, C).transpose(1, 0, 2)
        .reshape(128, 512))
    put(WV, (wv * sc[None, :]).T.reshape(2, 128, C).transpose(1, 0, 2)
        .reshape(128, 512))
    put(WO, wo.T.reshape(2, 128, C).transpose(1, 0, 2).reshape(128, 512))
    put(WQD2, wqd2.T)                      # [HID, C]
    smask, dmask, zmask = _make_masks()
    for n in range(NCOL):
        put(ZM + n * 128, zmask[n])
    put(WD1, wd1_eff)                      # [4, HID]
    put(SM, smask)
    put(DM, dmask)
    return _bf16(wts)


def kernel(ego_features, ego_demand, collaborator_features,
           w_d1, b_d1, w_d2, b_d2, wq, bq, wk, bk, wv, bv, wo, bo,
           pos_emb):
    ego_features = np.asarray(ego_features, np.float32)
    ego_demand = np.asarray(ego_demand, np.float32)
    collaborator_features = np.asarray(collaborator_features, np.float32)
    w_d1 = np.asarray(w_d1, np.float32); b_d1 = np.asarray(b_d1, np.float32)
    w_d2 = np.asarray(w_d2, np.float32); b_d2 = np.asarray(b_d2, np.float32)
    wq = np.asarray(wq, np.float32); bq = np.asarray(bq, np.float32)
    wk = np.asarray(wk, np.float32); bk = np.asarray(bk, np.float32)
    wv = np.asarray(wv, np.float32); bv = np.asarray(bv, np.float32)
    wo = np.asarray(wo, np.float32); bo = np.asarray(bo, np.float32)
    pos_emb = np.asarray(pos_emb, np.float32)

    # ---- quantize activations (per-channel linear int8) ----
    acts, se, sc = _quant_acts(ego_features[0], pos_emb[0],
                               collaborator_features)
    d8, sd = _chan_quant(ego_demand[0])                 # [3, H, W]

    # ---- fold scales + demand bias into weights ----
    scale = 1.0 / math.sqrt(HD)
    wq_s = wq * scale
    wqd2 = wq_s @ w_d2                                  # [C, HID]
    wd1_eff = np.zeros((4, HID), np.float32)
    wd1_eff[0:3] = (w_d1 * sd[None, :]).T
    wd1_eff[3] = b_d1 / 127.0
    wts = _pack_wts(wq_s, wk, wv, wo, wqd2, wd1_eff, se, sc)
    acts[:, :, WOFF:] = wts.view(np.int8)

    bq_eff = (bq + wq @ b_d2) * scale
    bo_eff = bo + wo @ bv
    has_bias = bool(np.any(bq_eff) or np.any(bo_eff))
    nc = _get_program(has_bias)
    if has_bias:
        bias_np = np.empty((128, 4), np.float32)
        bias_np[:, 0] = bq_eff[0:128]
        bias_np[:, 1] = bq_eff[128:256]
        bias_np[:, 2] = bo_eff[0:128]
        bias_np[:, 3] = bo_eff[128:256]

    d8f = np.empty((4, H, W), np.int8)
    d8f[0:3] = d8
    d8f[3] = 127
    in_maps = []
    for i in range(NCORES):
        sl = slice(i * HSL, (i + 1) * HSL)
        m = {"acts": acts[i], "dem": d8f[:, sl, :]}
        if has_bias:
            m["bias"] = bias_np
        in_maps.append(m)

    res = run_bass_kernel_spmd(nc, in_maps, list(range(NCORES)))

    # ---- dequantize + assemble ----
    out = np.empty((1, C, H, W), np.float32)
    for i in range(NCORES):
        oc = res.results[i]["out"]              # [2, 128, HSL*TP + 4] int8
        rrv = np.ascontiguousarray(oc[:, :, HSL * TP:]).view(np.float32)
        inv = (1.0 / rrv.reshape(C, 1, 1)).astype(np.float32)
        out[0, :, i * HSL:(i + 1) * HSL, :] = \
            oc[:, :, 0:HSL * TP].reshape(C, HSL, TP).astype(np.float32) * inv
    return out
